# revision 34
# baseline (speedup 1.0000x reference)
"""Dense GAT layer (nn_DenseGATLayer) Trainium2 Bass kernel, v4.

Problem (per batch b of B=8):
    Wh   = X[b] @ W                                   [N=1024, H*F=256]
    s[n,h] = <Wh[n,h,:], a_src[h]>,  d[n,h] = <Wh[n,h,:], a_dst[h]>
    e[i,j,h] = lrelu(s[i,h] + d[j,h], 0.2);  masked by A[b,i,j]
    alpha = softmax_j(e);  out[i,h,:] = elu(sum_j alpha[i,j,h] Wh[j,h,:])

Sharding: data-parallel, one batch per NeuronCore (B=8 == n_cores=8).

The N^2*H masked-softmax-numerator work (32 tiles of [128j, 1024i]) is
split across engines via a per-(head, jt-tile) MODE table:

  "E" (EXP):  DVE score op t = ATm + (s16 + d16); e = max(t, .2t)
              (int16 fixed-point, scale 2048, dual-pump 2x);
              Scalar exp -> bf16 P.
  "M" (MGP, separable): exp(lrelu(s_i+d_j)) = max(u_i v_j, p_i q_j)
              with u=e^s, v=e^d, p=e^{.2s}, q=e^{.2d}.
              GpSimd stt G = (p_rep * q_j) + ATm;
              DVE (dual-pump 2x) P = G>0 ? max(u_rep * v_j, G) : 0.

ATm[j,i] = (A^T-1)*1e6 in {0, -1e6} (bf16, host-prepped) is an
ADDITIVE mask for EXP and supplies the sign test for MGP (pq > 0
always, so unmasked G > 0).

Prologue: eT = Wa^T X^T on PE; s16 int16 rows + E1=exp(eT),
E2=exp(.2 eT) bf16 rows ride a DRAM round trip; stride-0 broadcast
DMAs replicate rows across partitions (S16REP for EXP heads,
UREP/PREP for MGP heads). Per-partition d-columns come from PE
transposes of 2048*eT (dcolsR) and Scalar exp of those (v/q cols).
Input DMA is spread over the sync/gpsimd/vector queues; the Scalar
engine issues no DMA. Normalize/elu for head h is emitted after head
h+1's scores so the DVE never stalls on the AV matmul tail.
"""

import sys

if "/opt/trn_rl_repo" not in sys.path:
    sys.path.insert(0, "/opt/trn_rl_repo")

from contextlib import ExitStack

import numpy as np

import concourse.bass as bass
import concourse.tile as tile
from concourse import bacc, mybir
from concourse import bass_utils
from concourse._compat import with_exitstack

# ------------------------------------------------------------------ params
B, N, DIN, H, F = 8, 1024, 256, 4, 64
HF = H * F
NT = N // 128            # 8 node tiles
KT = DIN // 128          # 2 contraction tiles
LRELU_ALPHA = 0.2
SCALE = 2048.0           # fixed-point scale for the log-space path
MBIG = 1.0e6             # additive mask magnitude (host bakes -MBIG into ATm)

dt = mybir.dt
AF = mybir.ActivationFunctionType
f32r = dt.float32r
ALU = mybir.AluOpType

# ------------------------------------------------------------- custom DVE ops
from concourse.dve_ops import (
    DveOp,
    OPS,
    _SUB_OPCODE_FOR_NAME,
    CUSTOM_DVE_SPECS,
    _CUSTOM_DVE_ROW_BASE,
)
from concourse.dve_spec import (
    Spec,
    Src0,
    Src1,
    C0,
    C1,
    Zero,
    One,
    lower,
    maxx,
    select,
    _has_src1,
)
from concourse.dve_uop import (
    DveOpSpec,
    UopConfig,
    UopDpConfig,
    InpSel,
    OutSel,
    OutPath,
    AluInp,
    DelayInp,
    Trigger,
    AluOp as UAluOp,
)

_KEEP = DelayInp.PREV_DELAY
_CAP = DelayInp.PREV_ALU_OUT
_DL = [AluInp.PREV_DELAY_0, AluInp.PREV_DELAY_1, AluInp.PREV_DELAY_2,
       AluInp.PREV_DELAY_3, AluInp.PREV_DELAY_4, AluInp.PREV_DELAY_5]
_PREV = AluInp.PREV_ALU_OUT


def _dp2(op, a, b, cap_lane=None):
    delay = [_KEEP] * 7
    if cap_lane is not None:
        delay[cap_lane] = _CAP
    return UopDpConfig(op=op, alu_src0=a, alu_src1=b, delay=delay,
                       alu_out_enable=1, swap_enable=0, alu_out_a_enable=0,
                       alu_out_b_enable=0, delay_enable=[1, 1, 1, 1, 1, 1, 0],
                       idx0_sel=0, idx1_sel=0)


def _uop2x(inp, dp):
    """Dual-pump UopConfig: LO chain stages 0-3, HI chain 4-7; the LO
    result is captured into lane 0 mid-chain and read via WR0_LO=DELAY_0;
    the HI result is the final ALU_OUT."""
    return UopConfig(
        inp=inp,
        inp_enable=[0, 1, 1, 1, 1, 1, 1, 0],
        out={OutPath.WR0_LO: OutSel.DELAY_0, OutPath.WR0_HI: OutSel.ALU_OUT,
             OutPath.WR1_LO: OutSel.ALU_OUT, OutPath.WR1_HI: OutSel.ALU_OUT},
        out_enable={OutPath.WR0_LO: 1, OutPath.WR0_HI: 1,
                    OutPath.WR1_LO: 0, OutPath.WR1_HI: 0},
        require_inp0=1, require_inp1=1,
        trigger=(Trigger.SRC_TENSOR_DONE, Trigger.NONE, Trigger.NONE),
        next_uop=(0, 0, 0), datapath_config=dp)


def build_score3_2x():
    """_t = Src0 + (Src1 + C0); max(_t, _t*C1) dual-pumped.
    Lanes: D0=Src0 D1=Src1 D2=C0 D3=C1 D4=Src0_HI D5=Src1_HI."""
    dp = [
        _dp2(UAluOp.ADD, _DL[1], _DL[2]),
        _dp2(UAluOp.ADD, _DL[0], _PREV),
        _dp2(UAluOp.MULTIPLY, _PREV, _DL[3], cap_lane=0),
        _dp2(UAluOp.MAX, _DL[0], _PREV),
        _dp2(UAluOp.ADD, _DL[5], _DL[2], cap_lane=0),
        _dp2(UAluOp.ADD, _DL[4], _PREV),
        _dp2(UAluOp.MULTIPLY, _PREV, _DL[3], cap_lane=1),
        _dp2(UAluOp.MAX, _DL[1], _PREV),
    ]
    inp = [InpSel.ZERO, InpSel.SRC_0, InpSel.SRC_1, InpSel.CONST_0,
           InpSel.CONST_1, InpSel.SRC_0_HI, InpSel.SRC_1_HI, InpSel.ZERO]
    return _uop2x(inp, dp)


def build_wmax_2x():
    """select(Src1 > 0, max(Src0*C0, Src1), 0) dual-pumped.
    Lanes: D0=Src0 D1=Src1 D2=C0 D3=ZERO D4=Src0_HI D5=Src1_HI.
    HW SELECT takes the cond from PREV_ALU_OUT and routes alu_src1 when
    truthy, alu_src0 when falsy."""
    dp = [
        _dp2(UAluOp.MULTIPLY, _DL[0], _DL[2]),              # uv_lo
        _dp2(UAluOp.MAX, _PREV, _DL[1]),                    # m_lo
        _dp2(UAluOp.IS_LT, _DL[3], _DL[1], cap_lane=0),     # cond; D0 <- m_lo
        _dp2(UAluOp.SELECT, _DL[3], _DL[0]),                # r_lo = c ? m : 0
        _dp2(UAluOp.MULTIPLY, _DL[4], _DL[2], cap_lane=0),  # uv_hi; D0 <- r_lo
        _dp2(UAluOp.MAX, _PREV, _DL[5]),                    # m_hi
        _dp2(UAluOp.IS_LT, _DL[3], _DL[5], cap_lane=1),     # cond; D1 <- m_hi
        _dp2(UAluOp.SELECT, _DL[3], _DL[1]),                # r_hi
    ]
    inp = [InpSel.ZERO, InpSel.SRC_0, InpSel.SRC_1, InpSel.CONST_0,
           InpSel.ZERO, InpSel.SRC_0_HI, InpSel.SRC_1_HI, InpSel.ZERO]
    return _uop2x(inp, dp)


def _dpass(cap_lane=None):
    """Pass-through stage (BYPASS forwards PREV_ALU_OUT, encoding 0 --
    same as the lowerer's trailing stages)."""
    return _dp2(UAluOp.BYPASS, _PREV, _PREV, cap_lane=cap_lane)


def build_sel2_2x():
    """select(Src0 >= 0, Src0, Src1 - 1) dual-pumped (elu tail).
    Lanes: D0=Src0 D1=Src1 D2=ONE D3=ZERO D4=Src0_HI D5=Src1_HI."""
    dp = [
        _dp2(UAluOp.SUBTRACT, _DL[1], _DL[2]),              # em1_lo
        _dp2(UAluOp.IS_GE, _DL[0], _DL[3], cap_lane=1),     # cond; D1 <- em1_lo
        _dp2(UAluOp.SELECT, _DL[1], _DL[0]),                # r_lo = c ? Src0 : em1
        _dp2(UAluOp.SUBTRACT, _DL[5], _DL[2], cap_lane=0),  # em1_hi; D0 <- r_lo
        _dp2(UAluOp.IS_GE, _DL[4], _DL[3], cap_lane=1),     # cond; D1 <- em1_hi
        _dp2(UAluOp.SELECT, _DL[1], _DL[4]),                # r_hi
        _dpass(),
        _dpass(),
    ]
    inp = [InpSel.ZERO, InpSel.SRC_0, InpSel.SRC_1, InpSel.ONE_F32,
           InpSel.ZERO, InpSel.SRC_0_HI, InpSel.SRC_1_HI, InpSel.ZERO]
    return _uop2x(inp, dp)


class _DveOp2x(DveOp):
    """DveOp whose table also carries a hand-built dual-pump 2x program
    (v3/TRN2 needs real pair-routed uops). perf_max=1 on the instruction
    makes the engine select the 2x slot when all non-scalar operands are
    packed 2-byte dtypes."""

    def __init__(self, name, spec, build2x, **kw):
        object.__setattr__(self, "_build2x", build2x)
        super().__init__(name, spec, **kw)

    def _uops2x(self, ver):
        return [self._build2x()] if ver == "v3" else lower(self.spec, ver=ver)

    def compile(self, ver):
        from concourse.dve_ops import _COMPILE_CACHE, get_dve_sub_opcode

        key = (self.name, ver)
        if (r := _COMPILE_CACHE.get(key)) is not None:
            return r
        result = DveOpSpec(
            name=self.name,
            opcode=get_dve_sub_opcode(self.name),
            uops=lower(self.spec, ver=ver),
            uops_2x=self._uops2x(ver),
            perf_max=1,
            rd1_en=_has_src1(self.spec),
        )
        got = result.sha(ver)
        if self.uops_sha.get(ver) != got:
            raise ValueError(f"{self.name}: sha drift {got}")
        _COMPILE_CACHE[key] = result
        return result


def _register_op_2x(name, spec, build2x):
    for o in OPS:
        if o.name == name:
            return o
    opcode = _CUSTOM_DVE_ROW_BASE + len(OPS)
    shas = {}
    for ver in ("v3", "v4"):
        sp = DveOpSpec(
            name=name,
            opcode=opcode,
            uops=lower(spec, ver=ver),
            uops_2x=[build2x()] if ver == "v3" else lower(spec, ver=ver),
            perf_max=1,
            rd1_en=_has_src1(spec),
        )
        shas[ver] = sp.sha(ver)
    op = _DveOp2x(name, spec, build2x, subdim=False, uops_sha=shas)
    OPS.append(op)
    _SUB_OPCODE_FOR_NAME[name] = opcode
    CUSTOM_DVE_SPECS[name] = spec
    return op


def _score3_ref(in0, in1, s0, s1, imm2):
    t = np.asarray(in0, np.float32) + np.asarray(in1, np.float32) + s0
    return np.maximum(t, t * s1)


# log-space masked leaky-relu score with bf16 additive mask (in0: 0/-1e6):
# t = in0 + in1 + s0; out = max(t, t*s1)
_t = Src0 + (Src1 + C0)
GAT_SCORE3 = _register_op_2x(
    "GAT_SCORE3_ANT",
    Spec(body=maxx(_t, _t * C1), reference=_score3_ref),
    build_score3_2x,
)

# separable pass B: P = in1 > 0 ? max(in0 * s0, in1) : 0
GAT_WMAX = _register_op_2x(
    "GAT_WMAX2_ANT",
    Spec(
        body=select(Src1 > Zero, maxx(Src0 * C0, Src1), Zero),
        reference=lambda in0, in1, s0, s1, imm2: np.where(
            np.asarray(in1, np.float32) > 0,
            np.maximum(np.asarray(in0, np.float32) * s0, np.asarray(in1, np.float32)),
            0.0,
        ),
    ),
    build_wmax_2x,
)

# elu select: out = in0 >= 0 ? in0 : in1 - 1
GAT_SEL2 = _register_op_2x(
    "GAT_SEL2B_ANT",
    Spec(
        body=select(Src0 >= Zero, Src0, Src1 - One),
        reference=lambda in0, in1, s0, s1, imm2: np.where(in0 >= 0, in0, in1 - 1),
    ),
    build_sel2_2x,
)


def _register_op_1x(name, spec):
    for o in OPS:
        if o.name == name:
            return o
    opcode = _CUSTOM_DVE_ROW_BASE + len(OPS)
    shas = {}
    for ver in ("v3", "v4"):
        s = DveOpSpec(
            name=name, opcode=opcode, uops=lower(spec, ver=ver),
            rd1_en=_has_src1(spec),
        )
        shas[ver] = s.sha(ver)
    op = DveOp(name, spec, subdim=False, uops_sha=shas)
    OPS.append(op)
    _SUB_OPCODE_FOR_NAME[name] = opcode
    CUSTOM_DVE_SPECS[name] = spec
    return op


def _pq5_ref(in0, in1, s0, s1, imm2):
    pq = np.asarray(in0, np.float32) * s0
    G = pq + np.asarray(in1, np.float32)
    return np.maximum(pq**5, G) * (G > 0)


# fused separable score: pq = in0*s0 (= e^{.2(s_i+d_j)}); G = pq + mask;
# P = max(pq^5, G) * (G > 0)   (uses u*v = (p*q)^5; 8 ALU stages, 1x)
_pq = Src0 * C0
_G = _pq + Src1
GAT_PQ5 = _register_op_1x(
    "GAT_PQ5_ANT",
    Spec(
        body=maxx(((_pq * _pq) * (_pq * _pq)) * _pq, _G) * (_G > Zero),
        reference=_pq5_ref,
    ),
)


def _bcast_last(ap, n):
    """Append a step-0 free dim of size n to an AP (broadcast along it)."""
    return bass.AP(ap.tensor, ap.offset, [list(d) for d in ap.ap] + [[0, n]])


def _bcast_part(ap, n):
    """Prepend a step-0 partition dim of size n (broadcast; DMA use only)."""
    return bass.AP(ap.tensor, ap.offset, [[0, n]] + [list(d) for d in ap.ap])


# per-(head, jt) pipeline mode:
#   "E" = EXP path (DVE score3 2x + Scalar exp)
#   "P" = fused separable (single DVE 1x PQ5 op; no Scalar)
#   "G" = GpSimd-assisted separable (GpSimd pq+mask passes + DVE WMAX 2x)
MODE = {0: "EEEEEEEE", 1: "EEEEEEEE", 2: "PPPPPPPP", 3: "EEEEEEEE"}
SREP = {h: i for i, h in enumerate(h for h in range(H) if "E" in MODE[h])}
PREPS = {h: i for i, h in enumerate(h for h in range(H)
                                    if "P" in MODE[h] or "G" in MODE[h])}
UREPS = {h: i for i, h in enumerate(h for h in range(H) if "G" in MODE[h])}
G_TILES = [(h, jt) for h in range(H) for jt in range(NT) if MODE[h][jt] == "G"]
GIDX = {hj: i for i, hj in enumerate(G_TILES)}
N_SREP = max(1, len(SREP))
N_PREP = max(1, len(PREPS))
N_UREP = max(1, len(UREPS))
SEL2_PERF = 1  # 1 enables the dual-pump elu select
DEBUG = 0      # 1 adds DET/DPT debug outputs


# ------------------------------------------------------------------ kernel body
@with_exitstack
def _gat_body(ctx: ExitStack, tc: "tile.TileContext", XTd, ATd, Wd, Wad, ID16d, OUTd,
              DETd=None, DPTd=None):
    nc = tc.nc
    f32, bf16, i16 = dt.float32, dt.bfloat16, dt.int16

    sb = ctx.enter_context(tc.tile_pool(name="sb", bufs=1))
    dram = ctx.enter_context(tc.tile_pool(name="dram", bufs=1, space="DRAM"))

    # ---------- input loads ---------------------------------------------------
    # queues: sync(SP) / gpsimd(Pool) / vector(DVE). Scalar issues no DMA.
    XTsb = sb.tile([128, KT * N], bf16)  # [p=din%128, kt, node]
    XTv = XTsb[:].rearrange("p (kt n) -> p kt n", kt=KT)
    XTdv = XTd[:].rearrange("(kt p) n -> p kt n", p=128)
    Wsb = sb.tile([128, KT * HF], bf16)
    Wasb = sb.tile([128, KT * 2 * H], bf16)
    NH = N // 2
    ident = sb.tile([16, 16], f32)
    # single-descriptor loads (each DMA_DIRECT2D costs ~0.7us of engine
    # issue time, so descriptor count is minimized and split by deadline)
    Wav = Wasb[:].rearrange("p (kt c) -> p kt c", kt=KT)
    Wadv = Wad[:].rearrange("(kt p) c -> p kt c", p=128)
    nc.sync.dma_start(Wav[:], Wadv[:])
    nc.sync.dma_start(ident[:], ID16d[:])
    nc.sync.dma_start(XTv[:, :, 0:NH], XTdv[:, :, 0:NH])
    nc.gpsimd.dma_start(XTv[:, :, NH:N], XTdv[:, :, NH:N])
    Wv = Wsb[:].rearrange("p (kt c) -> p kt c", kt=KT)
    Wdv = Wd[:].rearrange("(kt p) c -> p kt c", p=128)
    nc.sync.dma_start(Wv[:], Wdv[:])

    ATsb = sb.tile([128, NT * N], bf16)  # ATm tile jt at cols [jt*N, (jt+1)*N)
    # mask chunks split over the sync/gpsimd queues, earliest-needed first
    nc.gpsimd.dma_start(ATsb[:, 0 : 2 * N], ATd[:, 0 : 2 * N])
    nc.sync.dma_start(ATsb[:, 2 * N : 5 * N], ATd[:, 2 * N : 5 * N])
    nc.gpsimd.dma_start(ATsb[:, 5 * N : 8 * N], ATd[:, 5 * N : 8 * N])

    # warm the exp activation table and the PE p-state off the critical path
    scrap = sb.tile([1, 1], f32)
    nc.gpsimd.memset(scrap[:], 0.0)
    nc.scalar.activation(scrap[:], scrap[:], AF.Exp)
    with tc.tile_pool(name="psWarm", bufs=1, space="PSUM") as psWm:
        wrm = psWm.tile([16, 16], f32)
        for _ in range(4):
            nc.tensor.transpose(wrm[:], ident[:], ident[:])

    # ---------- score-vector prep ---------------------------------------------
    # eT rows (Wa col order): 2h = s_h, 2h+1 = d_h
    eT2048 = sb.tile([8, N], f32)        # 2048 * eT (fp32)
    s16d = sb.tile([8, N], i16)          # round(2048 * eT) int16 rows
    f16 = dt.float16
    E12 = sb.tile([8, 2 * N], f16)       # cols [0,N): exp(eT); [N,2N): exp(.2 eT)
    S16dr = dram.tile([8, N], i16)
    E12dr = dram.tile([8, 2 * N], f16)

    S16REP = sb.tile([128, N_SREP * N], i16)  # s16 replicated rows (E heads)
    UREP = sb.tile([128, N_UREP * N], f16)    # u=e^s replicated (G heads)
    PREP = sb.tile([128, N_PREP * N], f16)    # p=e^{.2s} replicated (P/G heads)
    dcolsR = sb.tile([128, NT * 8], f32)       # raw 2048*eT cols (d16 at col 2h+1)
    vcolR = sb.tile([128, NT * 8], f32)        # exp(eT) cols  (v at col 2h+1)
    qcolR = sb.tile([128, NT * 8], f32)        # exp(.2 eT) cols (q at col 2h+1)
    Whb = sb.tile([128, NT * H * (F + 1)], bf16)  # [p=node, jt, h, f|1]
    w4 = Whb[:].rearrange("p (jt h f) -> p jt h f", jt=NT, h=H)

    with (
        tc.tile_pool(name="psE", bufs=1, space="PSUM") as psE,
        tc.tile_pool(name="psT", bufs=2, space="PSUM") as psT,
        tc.tile_pool(name="psW", bufs=2, space="PSUM") as psW,
    ):
        # eT = Wa^T @ X^T  ([8, N] fp32)
        pe = psE.tile([8, N], f32)
        eT_mms = []
        for nh in range(2):
            for kt in range(KT):
                mi = nc.tensor.matmul(
                    pe[:, nh * NH : (nh + 1) * NH],
                    Wasb[:, kt * 2 * H : (kt + 1) * 2 * H],
                    XTsb[:, kt * N + nh * NH : kt * N + (nh + 1) * NH],
                    start=(kt == 0),
                    stop=(kt == KT - 1),
                )
                eT_mms.append(mi)
        # fixed-point rows by nh halves; the DRAM round trip starts as soon
        # as both halves retire (one descriptor).
        prep_acts = []
        for nh in range(2):
            cs = slice(nh * NH, (nh + 1) * NH)
            ai = nc.scalar.activation(s16d[:, cs], pe[:, cs], AF.Copy, scale=SCALE)
            prep_acts.append(ai)
            ai = nc.scalar.activation(eT2048[:, cs], pe[:, cs], AF.Copy, scale=SCALE)
            prep_acts.append(ai)
        nc.sync.dma_start(S16dr[:], s16d[:])
        # s16 rows for EXP heads replicated via stride-0 DRAM broadcast:
        # the first EXP head rides alone (earliest deadline), the rest share
        # one descriptor when their source rows are evenly strided.
        eheads = sorted(SREP)
        if eheads:
            h0 = eheads[0]
            nc.sync.dma_start(
                S16REP[:, 0:N], _bcast_part(S16dr[2 * h0, :], 128)
            )
            rest = eheads[1:]
            if rest:
                strides = {2 * (rest[i + 1] - rest[i]) for i in range(len(rest) - 1)}
                if len(rest) == 1 or len(strides) == 1:
                    st = strides.pop() * N if strides else 0
                    src = bass.AP(
                        S16dr[:].tensor,
                        S16dr[:].offset + 2 * rest[0] * N,
                        [[0, 128], [st, len(rest)], [1, N]],
                    )
                    nc.sync.dma_start(S16REP[:, N : (1 + len(rest)) * N], src)
                else:
                    for i, h in enumerate(rest):
                        nc.sync.dma_start(
                            S16REP[:, (1 + i) * N : (2 + i) * N],
                            _bcast_part(S16dr[2 * h, :], 128),
                        )

        # Wh (node-major bf16, strided per-head layout with ones column).
        # The w4 PSUM->SBUF copies land in the DVE's idle prologue window.
        nc.gpsimd.memset(w4[:, :, :, F], 1.0)
        # d columns for jt0-3 first (they gate the first score tiles)
        for jt in range(4):
            pd = psT.tile([128, 8], f32, tag="pt")
            sl = slice(jt * 128, (jt + 1) * 128)
            nc.tensor.transpose(pd[:], eT2048[:, sl], ident[0:8, 0:8])
            nc.vector.tensor_copy(dcolsR[:, jt * 8 : (jt + 1) * 8], pd[:])
        for it in range(NT):
            pw = psW.tile([128, HF], f32, tag="pw")
            for kt in range(KT):
                mi = nc.tensor.matmul(
                    pw[:],
                    XTsb[:, kt * N + it * 128 : kt * N + (it + 1) * 128],
                    Wsb[:, kt * HF : (kt + 1) * HF],
                    start=(kt == 0),
                    stop=(kt == KT - 1),
                )
                if it == 0:
                    for ei in eT_mms:
                        tile.add_dep_helper(mi.ins, ei.ins, reason="eT before Wh")
            nc.vector.tensor_copy(
                w4[:, it, :, 0:F], pw[:].rearrange("p (h f) -> p h f", h=H)
            )
        for jt in range(4, NT):
            pd = psT.tile([128, 8], f32, tag="pt")
            sl = slice(jt * 128, (jt + 1) * 128)
            nc.tensor.transpose(pd[:], eT2048[:, sl], ident[0:8, 0:8])
            nc.vector.tensor_copy(dcolsR[:, jt * 8 : (jt + 1) * 8], pd[:])

        if PREPS:
            # q columns + p-row round trip for the separable heads
            nc.scalar.activation(
                qcolR[:], dcolsR[:], AF.Exp, scale=LRELU_ALPHA / SCALE
            )
            for nh in range(2):
                cs = slice(nh * NH, (nh + 1) * NH)
                cs2 = slice(N + nh * NH, N + (nh + 1) * NH)
                nc.scalar.activation(E12[:, cs2], pe[:, cs], AF.Exp,
                                     scale=LRELU_ALPHA)
            nc.gpsimd.dma_start(E12dr[:, N : 2 * N], E12[:, N : 2 * N])
            for h in sorted(PREPS):
                k = PREPS[h]
                nc.gpsimd.dma_start(
                    PREP[:, k * N : (k + 1) * N],
                    _bcast_part(E12dr[2 * h, N : 2 * N], 128),
                )
        if UREPS:
            nc.scalar.activation(vcolR[:], dcolsR[:], AF.Exp, scale=1.0 / SCALE)
            for nh in range(2):
                cs = slice(nh * NH, (nh + 1) * NH)
                nc.scalar.activation(E12[:, cs], pe[:, cs], AF.Exp)
            nc.gpsimd.dma_start(E12dr[:, 0:N], E12[:, 0:N])
            for h in sorted(UREPS):
                k = UREPS[h]
                nc.gpsimd.dma_start(
                    UREP[:, k * N : (k + 1) * N], _bcast_part(E12dr[2 * h, 0:N], 128)
                )

    # ---------- GpSimd-assisted G tiles ----------------------------------------
    # For "G"-mode tiles the GpSimd computes G = pq + mask ahead of time
    # (tensor_scalar pq, then tensor_tensor +mask); the DVE consumes them
    # with the dual-pump WMAX much later, so GpSimd latency is hidden.
    NG = max(1, len(G_TILES))
    Gsb = sb.tile([128, NG * N], bf16)
    gtmp = ctx.enter_context(tc.tile_pool(name="gtmp", bufs=2))
    for gi, (h, jt) in enumerate(G_TILES):
        k = PREPS[h]
        col = slice(jt * 8 + 2 * h + 1, jt * 8 + 2 * h + 2)
        t = gtmp.tile([128, N], bf16, tag="gt")
        nc.gpsimd.tensor_scalar_mul(
            t[:], PREP[:, k * N : (k + 1) * N], qcolR[:, col]
        )
        nc.gpsimd.tensor_tensor(
            Gsb[:, gi * N : (gi + 1) * N], t[:],
            ATsb[:, jt * N : (jt + 1) * N], op=ALU.add,
        )

    # ---------- head loop ------------------------------------------------------
    OutSB = sb.tile([128, NT * HF], bf16)
    Out4 = OutSB[:].rearrange("p (it h f) -> p it h f", it=NT, h=H)
    psA = ctx.enter_context(tc.tile_pool(name="psA", bufs=2, space="PSUM"))
    pts = ctx.enter_context(tc.tile_pool(name="pts", bufs=2))
    eps = ctx.enter_context(tc.tile_pool(name="eps", bufs=3))
    small = ctx.enter_context(tc.tile_pool(name="small", bufs=3))

    def emit_scores(h, PTh):
        modes = MODE[h]
        # group consecutive E tiles into pairs for one exp act per pair
        jt = 0
        while jt < NT:
            col = slice(jt * 8 + 2 * h + 1, jt * 8 + 2 * h + 2)
            if modes[jt] == "E":
                run = 1
                if jt + 1 < NT and modes[jt + 1] == "E":
                    run = 2
                ep = eps.tile([128, 2 * N], dt.float16, tag="ep")
                k0 = SREP[h]
                for k in range(run):
                    j = jt + k
                    ki = nc.vector._custom_dve(
                        GAT_SCORE3,
                        out=ep[:, k * N : (k + 1) * N],
                        in0=ATsb[:, j * N : (j + 1) * N],
                        in1=S16REP[:, k0 * N : (k0 + 1) * N],
                        s0=dcolsR[:, j * 8 + 2 * h + 1 : j * 8 + 2 * h + 2],
                        s1=LRELU_ALPHA,
                    )
                    ki.ins.perf_max = 1
                nc.scalar.activation(
                    PTh[:, jt * N : (jt + run) * N],
                    ep[:, 0 : run * N],
                    AF.Exp,
                    scale=1.0 / SCALE,
                )
                jt += run
            elif modes[jt] == "P":
                k0 = PREPS[h]
                nc.vector._custom_dve(
                    GAT_PQ5,
                    out=PTh[:, jt * N : (jt + 1) * N],
                    in0=PREP[:, k0 * N : (k0 + 1) * N],
                    in1=ATsb[:, jt * N : (jt + 1) * N],
                    s0=qcolR[:, col],
                )
                jt += 1
            else:
                gi = GIDX[(h, jt)]
                k0 = UREPS[h]
                ki = nc.vector._custom_dve(
                    GAT_WMAX,
                    out=PTh[:, jt * N : (jt + 1) * N],
                    in0=UREP[:, k0 * N : (k0 + 1) * N],
                    in1=Gsb[:, gi * N : (gi + 1) * N],
                    s0=vcolR[:, col],
                )
                ki.ins.perf_max = 1
                jt += 1

    def emit_av(h, PTh):
        last = h == H - 1
        if last:
            # split accumulation: the jt0-3 half retires mid-stream, so only
            # the jt4-7 half plus one add trails the final score tile
            accA = psA.tile([128, NT, 128], f32, tag="acc")
            accB = psA.tile([128, NT, 128], f32, tag="accB")
            for it in range(NT):
                for jt in range(4):
                    nc.tensor.matmul(
                        accA[:, it, 0 : F + 1],
                        PTh[:, jt * N + it * 128 : jt * N + (it + 1) * 128],
                        w4[:, jt, h, :],
                        start=(jt == 0),
                        stop=(jt == 3),
                    )
            accAs = small.tile([128, NT, F + 1], f32, tag="accAs")
            nc.scalar.copy(accAs[:], accA[:, :, 0 : F + 1])
            for it in range(NT):
                for jt in range(4, NT):
                    nc.tensor.matmul(
                        accB[:, it, 0 : F + 1],
                        PTh[:, jt * N + it * 128 : jt * N + (it + 1) * 128],
                        w4[:, jt, h, :],
                        start=(jt == 4),
                        stop=(jt == NT - 1),
                    )
            return (accA, accB, accAs)
        acc8 = psA.tile([128, NT, 128], f32, tag="acc")
        for it in range(NT):
            for jt in range(NT):
                nc.tensor.matmul(
                    acc8[:, it, 0 : F + 1],
                    PTh[:, jt * N + it * 128 : jt * N + (it + 1) * 128],
                    w4[:, jt, h, :],
                    start=(jt == 0),
                    stop=(jt == NT - 1),
                )
        return acc8

    def emit_norm(h, acc, store=None):
        last = h == H - 1
        ngroups = 2 if last else 1
        gsz = NT // ngroups
        for g in range(ngroups):
            its = slice(g * gsz, (g + 1) * gsz)
            if last:
                accA, accB, accAs = acc
                accS = small.tile([128, gsz, F + 1], f32, tag="accS")
                nc.vector.tensor_tensor(
                    accS[:], accAs[:, its, :], accB[:, its, 0 : F + 1],
                    op=ALU.add,
                )
                accv, zcol = accS[:, :, 0:F], accS[:, :, F]
            else:
                accv, zcol = acc[:, its, 0:F], acc[:, its, F]
            rz = small.tile([128, gsz], f32, tag="rz")
            nc.vector.reciprocal(rz[:], zcol)
            u8 = small.tile([128, gsz, F], bf16, tag="u8")
            nc.vector.tensor_tensor(u8[:], accv, _bcast_last(rz[:], F), op=ALU.mult)
            ev8 = small.tile([128, gsz, F], bf16, tag="ev8")
            nc.scalar.activation(ev8[:], u8[:], AF.Exp)
            ki = nc.vector._custom_dve(
                GAT_SEL2, out=Out4[:, its, h, :], in0=u8[:], in1=ev8[:]
            )
            ki.ins.perf_max = SEL2_PERF
            if store is not None:
                store[g]()

    OutV = OutSB[:].rearrange("p (it c) -> p it c", it=NT)
    OutDV = OUTd[:].rearrange("(it p) c -> p it c", p=128)
    final_store = [
        lambda: nc.sync.dma_start(OutDV[:, 0 : NT // 2, :], OutV[:, 0 : NT // 2, :]),
        lambda: nc.gpsimd.dma_start(OutDV[:, NT // 2 : NT, :], OutV[:, NT // 2 : NT, :]),
    ]
    pending = None  # (h, acc) whose normalize is deferred one head
    for h in range(H):
        PTh = pts.tile([128, NT * N], bf16, tag="pt")
        emit_scores(h, PTh)
        if h == 0 and DPTd is not None:
            nc.sync.dma_start(DPTd[:], PTh[:])
        acc = emit_av(h, PTh)
        if pending is not None:
            emit_norm(*pending)
        pending = (h, acc)
    emit_norm(*pending, store=final_store)
    if DETd is not None:
        nc.sync.dma_start(DETd[:], eT2048[:])


# ------------------------------------------------------------------ build/run
_NC_CACHE = {}


def _build_nc():
    if "nc" in _NC_CACHE:
        return _NC_CACHE["nc"]
    nc = bacc.Bacc(
        "TRN2",
        target_bir_lowering=False,
        debug=False,
        enable_asserts=False,
        num_devices=B,
    )
    XTd = nc.dram_tensor("XT", [DIN, N], dt.bfloat16, kind="ExternalInput").ap()
    ATd = nc.dram_tensor("ATm", [128, NT * N], dt.bfloat16, kind="ExternalInput").ap()
    Wd = nc.dram_tensor("W", [DIN, HF], dt.bfloat16, kind="ExternalInput").ap()
    Wad = nc.dram_tensor("Wa", [DIN, 2 * H], dt.bfloat16, kind="ExternalInput").ap()
    ID16d = nc.dram_tensor("ID16", [16, 16], dt.float32, kind="ExternalInput").ap()
    OUTd = nc.dram_tensor("OUT", [N, HF], dt.bfloat16, kind="ExternalOutput").ap()
    dbg = {}
    if DEBUG:
        dbg["DETd"] = nc.dram_tensor("DET", [8, N], dt.float32,
                                     kind="ExternalOutput").ap()
        dbg["DPTd"] = nc.dram_tensor("DPT", [128, NT * N], dt.bfloat16,
                                     kind="ExternalOutput").ap()
    with tile.TileContext(nc) as tc:
        _gat_body(tc, XTd, ATd, Wd, Wad, ID16d, OUTd, **dbg)
    nc.compile()
    _NC_CACHE["nc"] = nc
    return nc


def _host_prep(W, a_src, a_dst):
    Wh_w = np.asarray(W, np.float32).reshape(DIN, H, F)
    Wa = np.empty((DIN, 2 * H), np.float32)
    Wa[:, 0::2] = np.einsum("khf,hf->kh", Wh_w, np.asarray(a_src, np.float32))
    Wa[:, 1::2] = np.einsum("khf,hf->kh", Wh_w, np.asarray(a_dst, np.float32))
    return Wa


def _run(X, A, W, a_src, a_dst, **spmd_kwargs):
    import ml_dtypes

    bf = ml_dtypes.bfloat16
    X = np.asarray(X, np.float32)
    XT = np.ascontiguousarray(X.transpose(0, 2, 1)).astype(bf)       # [B, DIN, N]
    A8 = ((np.asarray(A, np.float32).transpose(0, 2, 1) - 1.0) * MBIG).astype(bf)
    # partition-major relayout [p, jt, n]: one big-descriptor DMA per chunk
    A8 = np.ascontiguousarray(
        A8.reshape(B, NT, 128, N).transpose(0, 2, 1, 3).reshape(B, 128, NT * N)
    )
    W = np.ascontiguousarray(np.asarray(W, np.float32))
    Wa = _host_prep(W, a_src, a_dst).astype(bf)
    W = W.astype(bf)
    nc = _build_nc()
    id16 = np.eye(16, dtype=np.float32)
    in_maps = [
        {"XT": XT[b], "ATm": A8[b], "W": W, "Wa": Wa, "ID16": id16}
        for b in range(B)
    ]
    res = bass_utils.run_bass_kernel_spmd(
        nc, in_maps, core_ids=list(range(B)), **spmd_kwargs
    )
    out = np.stack([np.asarray(res.results[b]["OUT"]) for b in range(B)])
    return out.astype(np.float32), res


def kernel(X, A, W, a_src, a_dst):
    out, _ = _run(X, A, W, a_src, a_dst)
    return out


if __name__ == "__main__":
    rng = np.random.default_rng(0)
    out = kernel(
        X=rng.standard_normal((B, N, DIN)).astype(np.float32),
        A=rng.integers(0, 2, size=(B, N, N)).astype(np.int32),
        W=(rng.standard_normal((DIN, HF)) * 0.06).astype(np.float32),
        a_src=(rng.standard_normal((H, F)) * 0.17).astype(np.float32),
        a_dst=(rng.standard_normal((H, F)) * 0.17).astype(np.float32),
    )
    print(out.shape, out.dtype)
